# revision 8
# baseline (speedup 1.0000x reference)
"""Trainium2 Bass kernel for a dense transformer decoder layer.

Tensor-parallel over 8 NeuronCores: each core owns 4 q-heads, 1 kv-head and
a 1/8 slice of the FFN hidden dim. One on-device AllReduce after the
attention output projection (with x/8 folded in so the AR result is the
residual h2 directly); the final FFN partial sums are combined on the host.

Layout convention: activations are kept transposed as [feature, token] so the
contraction dim of every matmul is already on SBUF partitions. q/k head dims
are de-interleaved (x0 block then x1 block) so RoPE acts on 32-row blocks.
All matmul operands are float32r (TF32-like, full PE rate at N>=256).
"""
import numpy as np

import concourse.bass as bass
import concourse.bacc as bacc
import concourse.tile as tile
from concourse import mybir
from concourse.masks import make_identity
from concourse.tile_rust import add_dep_helper

F32 = mybir.dt.float32
F32R = mybir.dt.float32r
AF = mybir.ActivationFunctionType
OP = mybir.AluOpType

N_CORES = 8
EPS = 1e-5


def _cfg(S=2048, F=8192):
    B, E, HD = 2, 2048, 64
    T = B * S
    c = dict(B=B, S=S, E=E, F=F, HD=HD, T=T)
    c["KT_E"] = E // 128                 # k-tiles over E
    c["TCH"] = min(512, T)               # token chunk
    c["NCH"] = T // c["TCH"]
    c["QC"] = min(512, S)                # attention q chunk
    c["NQC"] = S // c["QC"]
    c["KT_S"] = S // 128                 # k-tiles per batch (attention)
    c["Fc"] = F // N_CORES               # FFN rows per core
    c["FH"] = 2                          # FFN f-half phases
    c["FHR"] = c["Fc"] // 2              # rows per half
    c["FHM"] = c["FHR"] // 128           # m-tiles / k-tiles per half
    assert c["FHM"] >= 1
    return c


def build(cfg):
    c = cfg
    E, T, TCH, NCH = c["E"], c["T"], c["TCH"], c["NCH"]
    KT_E, QC, NQC, KT_S = c["KT_E"], c["QC"], c["NQC"], c["KT_S"]
    B, S = c["B"], c["S"]
    FHR, FHM = c["FHR"], c["FHM"]
    QKT = QC // 128                      # k-tiles inside one diagonal q chunk

    nc = bacc.Bacc(None, target_bir_lowering=False, debug=False)

    # ---- I/O ----
    xT = nc.dram_tensor("xT", [E, T], F32R, kind="ExternalInput")
    x_tok = nc.dram_tensor("x_tok", [T, E], F32, kind="ExternalInput")
    wqkvT = nc.dram_tensor("wqkvT", [E, 448], F32R, kind="ExternalInput")
    woT = nc.dram_tensor("woT", [256, E], F32R, kind="ExternalInput")
    w1T = nc.dram_tensor("w1T", [E, c["Fc"]], F32R, kind="ExternalInput")
    w3T = nc.dram_tensor("w3T", [E, c["Fc"]], F32R, kind="ExternalInput")
    w2T = nc.dram_tensor("w2T", [c["Fc"], E], F32R, kind="ExternalInput")
    cosq = nc.dram_tensor("cosq", [128, T], F32, kind="ExternalInput")
    sinq = nc.dram_tensor("sinq", [128, T], F32, kind="ExternalInput")
    outT = nc.dram_tensor("outT", [E, T], F32, kind="ExternalOutput")

    replica_groups = [list(range(N_CORES))]

    with tile.TileContext(nc) as tc:
        with (
            tc.tile_pool(name="dram", bufs=1, space="DRAM") as dram,
            tc.tile_pool(name="dram_r", bufs=4, space="DRAM") as dram_r,
        ):
            o_bounce = dram.tile([E, T], F32)
            h2_sh = dram.tile([E, T], F32, addr_space="Shared")
            gT_dram = dram.tile([E, T], F32R)
            s1_dram = dram.tile([T], F32)
            s2_dram = dram.tile([T], F32)

            # manually-scoped pools with nested lifetimes: ao > qk > vt
            ao_cm = tc.tile_pool(name="ao", bufs=1)          # .. oproj end
            ao_pool = ao_cm.__enter__()
            qk_cm = tc.tile_pool(name="qk", bufs=1)          # .. attention end
            qk = qk_cm.__enter__()
            vt_cm = tc.tile_pool(name="vt", bufs=1)          # .. rope end
            vt_pool = vt_cm.__enter__()

            aoT0 = ao_pool.tile([128, T], F32R, tag="aoT0")
            aoT1 = ao_pool.tile([128, T], F32R, tag="aoT1")
            # q/k tiles: written by qkv matmul epilogue, roped in place.
            qr0 = qk.tile([128, T], F32R, tag="qr0")   # q heads 0,1
            qr1 = qk.tile([128, T], F32R, tag="qr1")   # q heads 2,3
            kr = qk.tile([128, T], F32R, tag="kr")     # kv head x2
            vaug = qk.tile([128, B * KT_S, 65], F32R, tag="vaug")
            vT = vt_pool.tile([64, T], F32R, tag="vT")

            # ---------- phase 0: norm1 scales ----------
            with (
                tc.tile_pool(name="p0", bufs=3) as p0,
                tc.tile_pool(name="p0s", bufs=4) as p0s,
            ):
                eps128 = p0.tile([128, 1], F32, tag="eps")
                nc.vector.memset(eps128[:], EPS)
                for i in range(T // 128):
                    xt = p0.tile([128, E], F32, tag="xt")
                    nc.sync.dma_start(out=xt[:], in_=x_tok[i * 128:(i + 1) * 128, :])
                    sq = p0.tile([128, E], F32, tag="sq")
                    ssum = p0s.tile([128, 1], F32, tag="ssum")
                    nc.scalar.activation(out=sq[:], in_=xt[:], func=AF.Square,
                                         accum_out=ssum[:])
                    rt = p0s.tile([128, 1], F32, tag="rt")
                    nc.scalar.activation(out=rt[:], in_=ssum[:], func=AF.Sqrt,
                                         scale=1.0 / E, bias=eps128[:])
                    r0 = p0s.tile([128, 1], F32, tag="r0")
                    nc.vector.reciprocal(out=r0[:], in_=rt[:])
                    # Newton: r = r0 * (2 - rt*r0)
                    t1 = p0s.tile([128, 1], F32, tag="t1")
                    nc.vector.tensor_tensor(out=t1[:], in0=rt[:], in1=r0[:], op=OP.mult)
                    nc.vector.tensor_scalar(out=t1[:], in0=t1[:], scalar1=-1.0,
                                            scalar2=2.0, op0=OP.mult, op1=OP.add)
                    rr = p0s.tile([128, 1], F32, tag="rr")
                    nc.vector.tensor_tensor(out=rr[:], in0=r0[:], in1=t1[:], op=OP.mult)
                    nc.sync.dma_start(out=s1_dram[i * 128:(i + 1) * 128], in_=rr[:])

            # ---------- phase 1: qkv projection ----------
            with (
                tc.tile_pool(name="qkvw", bufs=1) as qkvw,
                tc.tile_pool(name="qkvx", bufs=2) as qkvx,
                tc.tile_pool(name="qkvs", bufs=2) as qkvs,
                tc.tile_pool(name="qkv_ps", bufs=3, space="PSUM") as qkv_ps,
            ):
                wq_sb = qkvw.tile([128, KT_E, 448], F32R, tag="wq")
                for kt in range(KT_E):
                    nc.sync.dma_start(out=wq_sb[:, kt, :],
                                      in_=wqkvT[kt * 128:(kt + 1) * 128, :])
                for tch in range(NCH):
                    t0 = tch * TCH
                    xch = qkvx.tile([128, KT_E, TCH], F32R, tag="xch")
                    for kt in range(KT_E):
                        nc.sync.dma_start(
                            out=xch[:, kt, :],
                            in_=xT[kt * 128:(kt + 1) * 128, t0:t0 + TCH])
                    s1b = qkvs.tile([128, TCH], F32, tag="s1b")
                    nc.gpsimd.dma_start(
                        out=s1b[:],
                        in_=s1_dram[t0:t0 + TCH].unsqueeze(0).partition_broadcast(128))
                    for m, (dst, rows) in enumerate(
                            [(qr0, 128), (qr1, 128), (kr, 128), (vT, 64)]):
                        ps = qkv_ps.tile([128, TCH], F32, tag="mm")
                        for kt in range(KT_E):
                            nc.tensor.matmul(
                                ps[:rows, :],
                                wq_sb[:, kt, m * 128:m * 128 + rows],
                                xch[:, kt, :],
                                start=(kt == 0), stop=(kt == KT_E - 1))
                        nc.vector.tensor_tensor(
                            out=dst[:rows, t0:t0 + TCH], in0=ps[:rows, :],
                            in1=s1b[:rows, :], op=OP.mult)

            # ---------- phase 2: rope (in place) + V transpose ----------
            with (
                tc.tile_pool(name="rope", bufs=1) as rope,
                tc.tile_pool(name="tr_ps", bufs=2, space="PSUM") as tr_ps,
            ):
                cos_sb = rope.tile([128, T], F32, tag="cos")
                sin_sb = rope.tile([128, T], F32, tag="sin")
                nc.sync.dma_start(out=cos_sb[:], in_=cosq[:])
                nc.sync.dma_start(out=sin_sb[:], in_=sinq[:])
                for qt in (qr0, qr1, kr):
                    swp = rope.tile([128, T], F32, tag="swp")
                    for b0 in (0, 64):
                        nc.sync.dma_start(out=swp[b0:b0 + 32, :],
                                          in_=qt[b0 + 32:b0 + 64, :].bitcast(F32))
                        nc.sync.dma_start(out=swp[b0 + 32:b0 + 64, :],
                                          in_=qt[b0:b0 + 32, :].bitcast(F32))
                    tm = rope.tile([128, T], F32, tag="tm")
                    nc.vector.tensor_tensor(out=tm[:], in0=qt[:].bitcast(F32),
                                            in1=cos_sb[:], op=OP.mult)
                    um = rope.tile([128, T], F32, tag="um")
                    nc.vector.tensor_tensor(out=um[:], in0=swp[:], in1=sin_sb[:],
                                            op=OP.mult)
                    nc.vector.tensor_tensor(out=qt[:], in0=tm[:], in1=um[:],
                                            op=OP.add)

                # V: [64, T] -> V_aug [k, 65] tiles (col 64 = ones)
                ident_f = rope.tile([64, 64], F32, tag="ident_f")
                make_identity(nc, ident_f[:])
                ident = rope.tile([64, 64], F32R, tag="ident")
                nc.vector.tensor_copy(out=ident[:], in_=ident_f[:])
                ones_f = rope.tile([128, 1], F32, tag="ones_f")
                nc.vector.memset(ones_f[:], 1.0)
                for kt in range(B * KT_S):
                    pt = tr_ps.tile([128, 64], F32R, tag="tr")
                    nc.tensor.transpose(pt[:], vT[:, kt * 128:(kt + 1) * 128], ident[:])
                    nc.vector.tensor_copy(out=vaug[:, kt, 0:64], in_=pt[:])
                    nc.vector.tensor_copy(out=vaug[:, kt, 64:65], in_=ones_f[:])
            vt_cm.__exit__(None, None, None)

            # ---------- phase 3: attention ----------
            with (
                tc.tile_pool(name="att", bufs=1) as att,
                tc.tile_pool(name="atts", bufs=4) as atts,
                tc.tile_pool(name="att_ps", bufs=2, space="PSUM") as att_ps,
                tc.tile_pool(name="attv_ps", bufs=2, space="PSUM") as attv_ps,
            ):
                for (qtile, aoT) in [(qr0, aoT0), (qr1, aoT1)]:
                    for b in range(B):
                        for qc in range(NQC):
                            qs = b * S + qc * QC
                            n_kb = qc * QKT + QKT
                            expsA = att.tile([128, KT_S, QC], F32R, tag="expsA")
                            expsB = att.tile([128, KT_S, QC], F32R, tag="expsB")
                            exps = [expsA, expsB]
                            for kb in range(n_kb):
                                ksl = slice(b * S + kb * 128, b * S + kb * 128 + 128)
                                for h, ps_tag in ((0, "scA"), (1, "scB")):
                                    ps = att_ps.tile([128, QC], F32, tag=ps_tag)
                                    nc.tensor.matmul(
                                        ps[:],
                                        kr[h * 64:(h + 1) * 64, ksl],
                                        qtile[h * 64:(h + 1) * 64, qs:qs + QC],
                                        start=True, stop=True)
                                    nc.scalar.activation(
                                        out=exps[h][:, kb, :], in_=ps[:], func=AF.Exp)
                                    j = kb - qc * QKT
                                    if j >= 0:
                                        nc.gpsimd.affine_select(
                                            out=exps[h][:, kb, :],
                                            in_=exps[h][:, kb, :],
                                            compare_op=OP.is_ge,
                                            fill=0.0, base=-128 * j,
                                            pattern=[[1, QC]], channel_multiplier=-1)
                            for h in range(2):
                                po = attv_ps.tile([65, QC], F32, tag="attv")
                                for kb in range(n_kb):
                                    gkt = b * KT_S + kb
                                    nc.tensor.matmul(
                                        po[:], vaug[:, gkt, :], exps[h][:, kb, :],
                                        start=(kb == 0), stop=(kb == n_kb - 1))
                                # softmax denominators live in row 64
                                ssb = atts.tile([1, QC], F32, tag="ssb")
                                nc.vector.tensor_copy(out=ssb[:], in_=po[64:65, :])
                                r0 = atts.tile([1, QC], F32, tag="r0")
                                nc.vector.reciprocal(out=r0[:], in_=ssb[:])
                                t1 = atts.tile([1, QC], F32, tag="t1")
                                nc.vector.tensor_tensor(out=t1[:], in0=ssb[:],
                                                        in1=r0[:], op=OP.mult)
                                nc.vector.tensor_scalar(
                                    out=t1[:], in0=t1[:], scalar1=-1.0, scalar2=2.0,
                                    op0=OP.mult, op1=OP.add)
                                rr = atts.tile([1, QC], F32, tag="rr")
                                nc.vector.tensor_tensor(out=rr[:], in0=r0[:],
                                                        in1=t1[:], op=OP.mult)
                                rscr = dram_r.tile([QC], F32, tag="rscr")
                                nc.sync.dma_start(out=rscr[:], in_=rr[:])
                                rb = atts.tile([64, QC], F32, tag="rb")
                                nc.gpsimd.dma_start(
                                    out=rb[:],
                                    in_=rscr[:].unsqueeze(0).partition_broadcast(64))
                                nc.vector.tensor_tensor(
                                    out=aoT[h * 64:(h + 1) * 64, qs:qs + QC],
                                    in0=po[0:64, :], in1=rb[:], op=OP.mult)
            qk_cm.__exit__(None, None, None)

            # ---------- phase 4: output projection (+ x/8) ----------
            with (
                tc.tile_pool(name="opw", bufs=1) as opw,
                tc.tile_pool(name="opx", bufs=2) as opx,
                tc.tile_pool(name="opo", bufs=4) as opo,
                tc.tile_pool(name="op_ps", bufs=4, space="PSUM") as op_ps,
            ):
                wo_sb = opw.tile([128, 2, E], F32R, tag="wo")
                for kt in range(2):
                    nc.sync.dma_start(out=wo_sb[:, kt, :],
                                      in_=woT[kt * 128:(kt + 1) * 128, :])
                for tch in range(NCH):
                    t0 = tch * TCH
                    xch = opx.tile([128, KT_E, TCH], F32, tag="xch2")
                    for kt in range(KT_E):
                        nc.sync.dma_start(
                            out=xch[:, kt, :],
                            in_=xT[kt * 128:(kt + 1) * 128, t0:t0 + TCH].bitcast(F32))
                    for em in range(KT_E):
                        ps = op_ps.tile([128, TCH], F32, tag="mm")
                        for kt, ao_t in ((0, aoT0), (1, aoT1)):
                            nc.tensor.matmul(
                                ps[:], wo_sb[:, kt, em * 128:(em + 1) * 128],
                                ao_t[:, t0:t0 + TCH],
                                start=(kt == 0), stop=(kt == 1))
                        ob = opo.tile([128, TCH], F32, tag="ob")
                        nc.vector.scalar_tensor_tensor(
                            out=ob[:], in0=xch[:, em, :], scalar=1.0 / N_CORES,
                            in1=ps[:], op0=OP.mult, op1=OP.add)
                        nc.sync.dma_start(
                            out=o_bounce[em * 128:(em + 1) * 128, t0:t0 + TCH],
                            in_=ob[:])
            ao_cm.__exit__(None, None, None)

            # ---------- phase 5: AllReduce -> h2 ----------
            nc.gpsimd.collective_compute(
                "AllReduce", OP.add, replica_groups=replica_groups,
                ins=[o_bounce.opt()], outs=[h2_sh.opt()])

            # ---------- phase 6: norm2, gT, h2/8 -> outT ----------
            out_write_insts = {}
            with (
                tc.tile_pool(name="n2c", bufs=1) as n2c,
                tc.tile_pool(name="n2", bufs=2) as n2,
                tc.tile_pool(name="n2k", bufs=3) as n2k,
                tc.tile_pool(name="n2s", bufs=4) as n2s,
                tc.tile_pool(name="n2_ps", bufs=2, space="PSUM") as n2_ps,
            ):
                ones_fb = n2c.tile([128, 1], F32, tag="ones_fb")
                nc.vector.memset(ones_fb[:], 1.0)
                ones_sb = n2c.tile([128, 1], F32R, tag="ones")
                nc.vector.tensor_copy(out=ones_sb[:], in_=ones_fb[:])
                eps1 = n2c.tile([1, 1], F32, tag="eps1")
                nc.vector.memset(eps1[:], EPS)
                for tch in range(NCH):
                    t0 = tch * TCH
                    h2c = n2.tile([128, KT_E, TCH], F32, tag="h2c")
                    for kt in range(KT_E):
                        nc.sync.dma_start(
                            out=h2c[:, kt, :],
                            in_=h2_sh[kt * 128:(kt + 1) * 128, t0:t0 + TCH])
                    ps = n2_ps.tile([1, TCH], F32, tag="ones")
                    for kt in range(KT_E):
                        sqc = n2k.tile([128, TCH], F32R, tag="sqc")
                        nc.scalar.activation(out=sqc[:], in_=h2c[:, kt, :],
                                             func=AF.Square)
                        nc.tensor.matmul(ps[:], ones_sb[:], sqc[:],
                                         start=(kt == 0), stop=(kt == KT_E - 1))
                    st = n2s.tile([1, TCH], F32, tag="st")
                    nc.scalar.activation(out=st[:], in_=ps[:], func=AF.Sqrt,
                                         scale=1.0 / E, bias=eps1[:])
                    r0 = n2s.tile([1, TCH], F32, tag="r0")
                    nc.vector.reciprocal(out=r0[:], in_=st[:])
                    t1 = n2s.tile([1, TCH], F32, tag="t1")
                    nc.vector.tensor_tensor(out=t1[:], in0=st[:], in1=r0[:], op=OP.mult)
                    nc.vector.tensor_scalar(out=t1[:], in0=t1[:], scalar1=-1.0,
                                            scalar2=2.0, op0=OP.mult, op1=OP.add)
                    rr = n2s.tile([1, TCH], F32, tag="rr")
                    nc.vector.tensor_tensor(out=rr[:], in0=r0[:], in1=t1[:], op=OP.mult)
                    nc.sync.dma_start(out=s2_dram[t0:t0 + TCH], in_=rr[:])
                    s2b = n2s.tile([128, TCH], F32, tag="s2b")
                    nc.gpsimd.dma_start(
                        out=s2b[:],
                        in_=s2_dram[t0:t0 + TCH].unsqueeze(0).partition_broadcast(128))
                    for kt in range(KT_E):
                        gc = n2k.tile([128, TCH], F32R, tag="gc")
                        nc.vector.tensor_tensor(out=gc[:], in0=h2c[:, kt, :],
                                                in1=s2b[:], op=OP.mult)
                        nc.sync.dma_start(
                            out=gT_dram[kt * 128:(kt + 1) * 128, t0:t0 + TCH],
                            in_=gc[:])
                        oh = n2k.tile([128, TCH], F32, tag="oh")
                        nc.scalar.mul(oh[:], h2c[:, kt, :], 1.0 / N_CORES)
                        wi = nc.sync.dma_start(
                            out=outT[kt * 128:(kt + 1) * 128, t0:t0 + TCH],
                            in_=oh[:])
                        out_write_insts[(kt, tch)] = wi.ins

            # ---------- phase 7: FFN (two f-half passes) ----------
            for fh in range(c["FH"]):
                f0 = fh * FHR
                with (
                    tc.tile_pool(name=f"ffw{fh}", bufs=1) as ffw,
                    tc.tile_pool(name=f"ffg{fh}", bufs=2) as ffg,
                    tc.tile_pool(name=f"ffh{fh}", bufs=2) as ffh,
                    tc.tile_pool(name=f"ffo{fh}", bufs=4) as ffo,
                    tc.tile_pool(name=f"ff_ps{fh}", bufs=2, space="PSUM") as ff_ps,
                    tc.tile_pool(name=f"ffd_ps{fh}", bufs=2, space="PSUM") as ffd_ps,
                ):
                    w1h = ffw.tile([128, KT_E, FHR], F32R, tag="w1h")
                    w3h = ffw.tile([128, KT_E, FHR], F32R, tag="w3h")
                    w2h = ffw.tile([128, FHM, E], F32R, tag="w2h")
                    for kt in range(KT_E):
                        nc.sync.dma_start(out=w1h[:, kt, :],
                                          in_=w1T[kt * 128:(kt + 1) * 128, f0:f0 + FHR])
                        nc.sync.dma_start(out=w3h[:, kt, :],
                                          in_=w3T[kt * 128:(kt + 1) * 128, f0:f0 + FHR])
                    for kf in range(FHM):
                        nc.sync.dma_start(
                            out=w2h[:, kf, :],
                            in_=w2T[f0 + kf * 128:f0 + (kf + 1) * 128, :])
                    for tch in range(NCH):
                        t0 = tch * TCH
                        gch = ffg.tile([128, KT_E, TCH], F32R, tag="gch")
                        for kt in range(KT_E):
                            nc.sync.dma_start(
                                out=gch[:, kt, :],
                                in_=gT_dram[kt * 128:(kt + 1) * 128, t0:t0 + TCH])
                        hff = ffh.tile([128, FHM, TCH], F32R, tag="hff")
                        for fm in range(FHM):
                            ps1 = ff_ps.tile([128, TCH], F32, tag="up1")
                            for kt in range(KT_E):
                                nc.tensor.matmul(
                                    ps1[:], w1h[:, kt, fm * 128:(fm + 1) * 128],
                                    gch[:, kt, :],
                                    start=(kt == 0), stop=(kt == KT_E - 1))
                            h1 = ffh.tile([128, TCH], F32, tag="h1")
                            nc.scalar.activation(out=h1[:], in_=ps1[:], func=AF.Silu)
                            ps3 = ff_ps.tile([128, TCH], F32, tag="up3")
                            for kt in range(KT_E):
                                nc.tensor.matmul(
                                    ps3[:], w3h[:, kt, fm * 128:(fm + 1) * 128],
                                    gch[:, kt, :],
                                    start=(kt == 0), stop=(kt == KT_E - 1))
                            nc.vector.tensor_tensor(out=hff[:, fm, :], in0=h1[:],
                                                    in1=ps3[:], op=OP.mult)
                        for em in range(KT_E):
                            psd = ffd_ps.tile([128, TCH], F32, tag="down")
                            for kf in range(FHM):
                                nc.tensor.matmul(
                                    psd[:], w2h[:, kf, em * 128:(em + 1) * 128],
                                    hff[:, kf, :],
                                    start=(kf == 0), stop=(kf == FHM - 1))
                            od = ffo.tile([128, TCH], F32, tag="od")
                            nc.any.tensor_copy(out=od[:], in_=psd[:])
                            ai = nc.gpsimd.dma_start(
                                out=outT[em * 128:(em + 1) * 128, t0:t0 + TCH],
                                in_=od[:], accum_op=OP.add)
                            add_dep_helper(ai.ins, out_write_insts[(em, tch)],
                                           reason="outT accumulate after base write")
                            out_write_insts[(em, tch)] = ai.ins

    if not nc.is_finalized():
        nc.finalize()
    return nc


# ---------------------------------------------------------------------------
# host side
# ---------------------------------------------------------------------------

_DEINT = np.r_[np.arange(0, 64, 2), np.arange(1, 64, 2)]


def _prep_inputs(x, freqs_cis, w_qkv, w_o, w1, w2, w3, attn_norm_w, ff_norm_w, cfg):
    c = cfg
    B, S, E, F, T = c["B"], c["S"], c["E"], c["F"], c["T"]
    H, KH, HD = 32, 8, 64
    KV = KH * HD

    x2 = np.asarray(x, dtype=np.float32).reshape(T, E)
    xT = np.ascontiguousarray(x2.T)

    fc = np.asarray(freqs_cis, dtype=np.float32)       # [S, 32, 2]
    cos32 = np.ascontiguousarray(fc[:, :, 0].T)        # [32, S]
    sin32 = np.ascontiguousarray(fc[:, :, 1].T)
    cosb = np.concatenate([cos32] * B, axis=1)         # [32, T]
    sinb = np.concatenate([sin32] * B, axis=1)
    cosq = np.tile(cosb, (4, 1))                       # [128, T]
    sinq = np.concatenate([-sinb, sinb, -sinb, sinb], axis=0)

    n1 = np.asarray(attn_norm_w, dtype=np.float32)
    n2 = np.asarray(ff_norm_w, dtype=np.float32)
    wq = np.asarray(w_qkv[:E], dtype=np.float32).reshape(H, HD, E)
    wk = np.asarray(w_qkv[E:E + KV], dtype=np.float32).reshape(KH, HD, E)
    wv = np.asarray(w_qkv[E + KV:], dtype=np.float32).reshape(KH, HD, E)
    w_o = np.asarray(w_o, dtype=np.float32)
    w1 = np.asarray(w1, dtype=np.float32)
    w3 = np.asarray(w3, dtype=np.float32)
    w2 = np.asarray(w2, dtype=np.float32)

    in_maps = []
    Fc = F // N_CORES
    for core in range(N_CORES):
        rows = []
        for j in range(4):
            rows.append(wq[core * 4 + j][_DEINT] * 0.125)
        kd = wk[core][_DEINT]
        rows += [kd, kd, wv[core]]
        wsh = np.concatenate(rows, axis=0) * n1[None, :]        # [448, E]
        wqkvT_np = np.ascontiguousarray(wsh.T)                  # [E, 448]
        woT_np = np.ascontiguousarray(w_o[:, core * 256:(core + 1) * 256].T)
        fsl = slice(core * Fc, (core + 1) * Fc)
        w1T_np = np.ascontiguousarray((w1[fsl] * n2[None, :]).T)  # [E, Fc]
        w3T_np = np.ascontiguousarray((w3[fsl] * n2[None, :]).T)
        w2T_np = np.ascontiguousarray(w2[:, fsl].T)               # [Fc, E]
        in_maps.append({
            "xT": xT, "x_tok": x2, "wqkvT": wqkvT_np, "woT": woT_np,
            "w1T": w1T_np, "w3T": w3T_np, "w2T": w2T_np,
            "cosq": cosq, "sinq": sinq,
        })
    return in_maps


_BUILD_CACHE = {}


def _get_nc(cfg_key):
    if cfg_key not in _BUILD_CACHE:
        _BUILD_CACHE[cfg_key] = build(_cfg(*cfg_key))
    return _BUILD_CACHE[cfg_key]


def run(x, freqs_cis, w_qkv, w_o, w1, w2, w3, attn_norm_w, ff_norm_w,
        S=2048, F=8192):
    from concourse.bass_utils import run_bass_kernel_spmd
    cfg = _cfg(S, F)
    in_maps = _prep_inputs(x, freqs_cis, w_qkv, w_o, w1, w2, w3,
                           attn_norm_w, ff_norm_w, cfg)
    nc = _get_nc((S, F))
    res = run_bass_kernel_spmd(nc, in_maps, core_ids=list(range(N_CORES)))
    acc = np.zeros((cfg["E"], cfg["T"]), dtype=np.float64)
    for r in res.results:
        acc += r["outT"].astype(np.float64)
    out = acc.T.reshape(cfg["B"], S, cfg["E"]).astype(np.float32)
    return out


def kernel(x, attention_mask, freqs_cis, w_qkv, w_o, w1, w2, w3,
           attn_norm_w, ff_norm_w):
    return run(x, freqs_cis, w_qkv, w_o, w1, w2, w3, attn_norm_w, ff_norm_w,
               S=2048, F=8192)
